# revision 14
# baseline (speedup 1.0000x reference)
"""MLA (Multi-head Latent Attention) fused Bass kernel for 8 TRN2 NeuronCores.

Sharding: core c = 2*b + j handles batch b and a 512-token query slice
(j=0 -> token chunks {0,3} of 256, j=1 -> chunks {1,2}; interleaving
balances causal-attention work). Each core emits complete output rows
for its own (batch, query-slice).

v2 changes vs the original baseline (which replicated all weights to all
8 cores as runtime inputs):
  - All weight matrices are embedded in the NEFF as inline Const tensors
    (loaded to HBM once at model-load time, not streamed per execution).
    kernel() hashes the weight arrays and rebuilds the NEFF if they change.
  - x is shipped only as each core's own 512-token slice ([C, 512] bf16).
    The kv-latent for the peer's 512 tokens comes from an on-device
    AllGather between core pairs (0.5 MB over on-chip links) after each
    core down-projects its own tokens; this also removes the duplicated
    kv down-projection within a pair.
  - kv/s ordering is the gathered order: global token chunks [0,3,1,2]
    (even core's interleaved chunks first, then odd core's). Causal masks
    and k-rope tables are host-generated in this order, so the compute
    is order-agnostic. Wide score tiles are s-tiles {0,1,4,5} (the ones
    that can touch the first query chunk), narrow are {2,3,6,7}.
  - The output is written in bf16 (host casts back to fp32).

All heavy matmuls run in bf16 (1 cycle/row on the PE) with fp32 PSUM
accumulation. Activations are kept in transposed layout ([feature, token])
so no on-chip transposes are needed anywhere:
  - scores are computed as scoresT[s, tq] = sum_d k[s,d] q[tq,d]
  - softmax skips max-subtraction (scores ~ N(0,1), exp is safe in fp32)
  - the denominator comes from an extra ones-column in V (fits in the
    same M<=128 PV matmuls), normalization is fused into PSUM eviction
  - causal masking is a 0/1 multiply with host-provided mask tiles
  - RoPE uses host-permuted (de-interleaved) rope weights so the rotation
    becomes out = x*C + swap32(x)*S with a PE permutation matmul for swap32
"""

import hashlib
import math
import os
import sys

sys.path.insert(0, "/opt/trn_rl_repo")

import ml_dtypes
import numpy as np

import concourse.bass as bass  # noqa: F401  (import keeps bass registered)
import concourse.mybir as mybir
import concourse.tile as tile
from concourse import bacc
from concourse.bass_utils import run_bass_kernel_spmd

BF = mybir.dt.bfloat16
F32 = mybir.dt.float32
NPBF = ml_dtypes.bfloat16

B, T, C = 4, 1024, 2048
H, DN, DR = 16, 128, 64
D = DN + DR  # 192
QR, KVR = 1536, 512
ROPE_BASE = 10000.0
NCORES = 8
TQ = 512          # query tokens per core
CH = 256          # tq chunk size
NST = 8           # s-tiles of 128
SCALE = 1.0 / math.sqrt(D)
SHUF = list(range(16, 32)) + list(range(0, 16))  # swap 16-row halves per 32-block

WIDE_STS = (0, 1, 4, 5)     # s-tiles that can reach query block A (cols 0:256)
NARROW_STS = (2, 3, 6, 7)
WI = {st: i for i, st in enumerate(WIDE_STS)}
NI = {st: i for i, st in enumerate(NARROW_STS)}

W_NAMES = (
    "wq_down", "wq_nope", "wq_rope", "wkv_down",
    "wv_up", "wk_nope", "wk_rope", "wo",
)

_CACHED_NC = None
_CACHED_KEY = None


def build_nc(w):
    """w: dict of host-prepared bf16 weight arrays (rope cols de-interleaved)."""
    nc = bacc.Bacc(
        "TRN2",
        target_bir_lowering=False,
        debug=False,
        enable_asserts=True,
        num_devices=NCORES,
    )

    # ---- runtime per-core inputs ----
    d_xq = nc.dram_tensor("xq", [C, TQ], BF, kind="ExternalInput")
    d_dl = nc.dram_tensor("dl", [1, 14], F32, kind="ExternalInput")
    d_out = nc.dram_tensor("out", [TQ, C], BF, kind="ExternalOutput")

    # k-side rope tables are identical on every core (S_PERM order) -> consts;
    # rc[r, c] = r - c feeds the on-device causal-mask build
    Ck, Sk = _rope_tables()
    d_ck = nc.inline_tensor(np.ascontiguousarray(Ck[:, S_PERM]).astype(NPBF), "ckc")
    d_sk = nc.inline_tensor(np.ascontiguousarray(Sk[:, S_PERM]).astype(NPBF), "skc")
    rc_np = (np.arange(128)[:, None] - np.arange(256)[None, :]).astype(NPBF)
    d_rc = nc.inline_tensor(rc_np, "rc")

    # ---- weights: NEFF-embedded constants ----
    d_wqd = nc.inline_tensor(w["wqd"], "wqd")   # [C, QR]
    d_wkd = nc.inline_tensor(w["wkd"], "wkd")   # [C, KVR]
    d_wqn = nc.inline_tensor(w["wqn"], "wqn")   # [QR, H*DN]
    d_wqr = nc.inline_tensor(w["wqr"], "wqr")   # [QR, H*DR] (de-interleaved)
    d_wkn = nc.inline_tensor(w["wkn"], "wkn")   # [KVR, H*DN]
    d_wkr = nc.inline_tensor(w["wkr"], "wkr")   # [KVR, H*DR] (de-interleaved)
    d_wv = nc.inline_tensor(w["wv"], "wv")      # [KVR, H*D]
    d_wo = nc.inline_tensor(w["wo"], "wo")      # [H*D, C]

    EXP = mybir.ActivationFunctionType.Exp
    MULT = mybir.AluOpType.mult

    with tile.TileContext(nc) as tc:
        with (
            tc.tile_pool(name="const", bufs=1) as constp,
            tc.tile_pool(name="lat", bufs=1) as latp,
            tc.tile_pool(name="attnout", bufs=1) as attnp,
            tc.tile_pool(name="psmm", bufs=3, space="PSUM") as psmm,
            tc.tile_pool(name="pss", bufs=2, space="PSUM") as pssp,
            tc.tile_pool(name="pspv", bufs=1, space="PSUM") as pspv,
            tc.tile_pool(name="wpair", bufs=2) as wp,
            tc.tile_pool(name="dram", bufs=1, space="DRAM") as dramp,
        ):
            # constants
            cq = constp.tile([128, TQ], BF)
            sq = constp.tile([128, TQ], BF)
            ck = constp.tile([128, T], BF)
            sk = constp.tile([128, T], BF)
            nc.sync.dma_start(ck[:], d_ck[:])
            nc.sync.dma_start(sk[:], d_sk[:])

            # causal masks built on device: block i is (rc <= delta_i), with
            # flat layout [W0A W0B W1A W1B W2A W2B W3A W3B | N0B N1B N2B N3B]
            rc_sb = constp.tile([128, 256], BF)
            dl_sb = constp.tile([128, 14], F32)
            mAB = constp.tile([128, 12 * 256], BF)
            nc.sync.dma_start(rc_sb[:], d_rc[:])
            nc.sync.dma_start(dl_sb[:], d_dl[:])
            for i in range(12):
                nc.vector.tensor_scalar(
                    mAB[:, i * 256:(i + 1) * 256], rc_sb[:],
                    dl_sb[:, i:i + 1], None, mybir.AluOpType.is_le,
                )

            # q-side rope tables: per-core half-select out of the const
            # k-tables (S_PERM order puts each core's own 512 cols in one
            # half: even cores 0:512, odd cores 512:1024)
            selt = constp.tile([128, TQ], BF)
            nc.vector.tensor_scalar(
                cq[:], ck[:, 0:TQ], dl_sb[:, 12:13], None, MULT)
            nc.vector.tensor_scalar(
                selt[:], ck[:, TQ:T], dl_sb[:, 13:14], None, MULT)
            nc.vector.tensor_add(cq[:], cq[:], selt[:])
            nc.vector.tensor_scalar(
                sq[:], sk[:, 0:TQ], dl_sb[:, 12:13], None, MULT)
            nc.vector.tensor_scalar(
                selt[:], sk[:, TQ:T], dl_sb[:, 13:14], None, MULT)
            nc.vector.tensor_add(sq[:], sq[:], selt[:])

            # persistent activations
            q_lat = latp.tile([128, QR // 128, TQ], BF)      # [r%128, rt, tq]
            kv_lat = latp.tile([128, KVR // 128, T], BF)     # [r%128, rt, s]

            # ---- Phase 1: latents ----
            with tc.tile_pool(name="ph1", bufs=1) as ph1:
                xq_sb = ph1.tile([128, 16, TQ], BF)
                wkd_sb = ph1.tile([128, 16, KVR], BF)
                kv_own = ph1.tile([128, KVR // 128, TQ], BF)
                r_xq = d_xq.rearrange("(k p) n -> p k n", p=128)
                r_wqd = d_wqd.rearrange("(k p) n -> p k n", p=128)
                r_wkd = d_wkd.rearrange("(k p) n -> p k n", p=128)
                for kt in range(16):
                    nc.sync.dma_start(wkd_sb[:, kt, :], r_wkd[:, kt, :])
                    nc.sync.dma_start(xq_sb[:, kt, :], r_xq[:, kt, :])

                # kv latent for own 512 tokens
                for rt in range(KVR // 128):
                    psk = psmm.tile([128, 512], F32, tag="mm", bufs=3)
                    for kt in range(16):
                        nc.tensor.matmul(
                            psk[:],
                            lhsT=wkd_sb[:, kt, rt * 128:(rt + 1) * 128],
                            rhs=xq_sb[:, kt, :],
                            start=(kt == 0),
                            stop=(kt == 15),
                        )
                    nc.vector.tensor_copy(kv_own[:, rt, :], psk[:])

                # pair AllGather: [own 512 | peer 512] in fixed (even, odd) order
                kvb = dramp.tile([128, KVR // 128, TQ], BF)
                gath = dramp.tile([2, 128, KVR // 128, TQ], BF)
                nc.sync.dma_start(kvb[:], kv_own[:])
                if os.environ.get("MLA_SIM_NOCC"):
                    # TimelineSim can't model collectives; stand in two DRAM
                    # copies with the same buffers so deps/timing stay close
                    nc.sync.dma_start(gath[0], kvb[:])
                    nc.sync.dma_start(gath[1], kvb[:])
                else:
                    nc.gpsimd.collective_compute(
                        "AllGather",
                        mybir.AluOpType.bypass,
                        replica_groups=[[0, 1], [2, 3], [4, 5], [6, 7]],
                        ins=[kvb.opt()],
                        outs=[gath.opt()],
                    )
                nc.sync.dma_start(kv_lat[:, :, 0:TQ], gath[0])
                nc.sync.dma_start(kv_lat[:, :, TQ:T], gath[1])

                # q latent (overlaps with the collective)
                for quarter in range(4):
                    wqd_q = ph1.tile([128, 16, 384], BF, tag="wqd_q", bufs=2)
                    nc.sync.dma_start(
                        wqd_q[:], r_wqd[:, :, quarter * 384:(quarter + 1) * 384]
                    )
                    for rtl in range(3):
                        rt = quarter * 3 + rtl
                        psq = psmm.tile([128, 512], F32, tag="mm", bufs=3)
                        for kt in range(16):
                            nc.tensor.matmul(
                                psq[:],
                                lhsT=wqd_q[:, kt, rtl * 128:(rtl + 1) * 128],
                                rhs=xq_sb[:, kt, :],
                                start=(kt == 0),
                                stop=(kt == 15),
                            )
                        nc.vector.tensor_copy(q_lat[:, rt, :], psq[:])

            # ---- Phase 2: per head-pair up-projections + attention ----
            with (
                tc.tile_pool(name="hwork", bufs=2) as hw,
                tc.tile_pool(name="probs", bufs=3) as prp,
                tc.tile_pool(name="small", bufs=2) as smp,
            ):
                outacc = attnp.tile([128, 4, C], F32, name="outacc")  # [t%128, tt, c]

                def emit_wo(sp_idx, attn_t, wo_t):
                    # one eviction per two head-pairs (6 kb blocks)
                    for tt in range(4):
                        for cch in range(4):
                            pso = psmm.tile(
                                [128, 512], F32, tag="wo", bufs=1, name="pso"
                            )
                            for kb in range(6):
                                nc.tensor.matmul(
                                    pso[:],
                                    lhsT=attn_t[:, kb, tt * 128:(tt + 1) * 128],
                                    rhs=wo_t[:, kb, cch * 512:(cch + 1) * 512],
                                    start=(kb == 0),
                                    stop=(kb == 5),
                                )
                            osl = outacc[:, tt, cch * 512:(cch + 1) * 512]
                            if sp_idx == 0:
                                nc.vector.tensor_copy(osl, pso[:])
                            else:
                                nc.vector.tensor_add(osl, osl, pso[:])
                        if sp_idx == H // 4 - 1:
                            ob = hw.tile([128, C], BF, tag="obf", bufs=2)
                            nc.vector.tensor_copy(ob[:], outacc[:, tt, :])
                            nc.sync.dma_start(
                                d_out[tt * 128:(tt + 1) * 128, :], ob[:]
                            )

                for p in range(H // 2):
                    # pair weight slabs
                    wqn_p = wp.tile([128, 12, 256], BF, tag="wqn_p")
                    wqr_p = wp.tile([128, 12, 128], BF, tag="wqr_p")
                    wkn_p = wp.tile([128, 4, 256], BF, tag="wkn_p")
                    wkr_p = wp.tile([128, 4, 128], BF, tag="wkr_p")
                    wv_p = wp.tile([128, 4, 384], BF, tag="wv_p")
                    nc.sync.dma_start(
                        wqn_p[:], d_wqn.rearrange("(k p) n -> p k n", p=128)[:, :, p * 256:(p + 1) * 256]
                    )
                    nc.sync.dma_start(
                        wqr_p[:], d_wqr.rearrange("(k p) n -> p k n", p=128)[:, :, p * 128:(p + 1) * 128]
                    )
                    nc.sync.dma_start(
                        wkn_p[:], d_wkn.rearrange("(k p) n -> p k n", p=128)[:, :, p * 256:(p + 1) * 256]
                    )
                    nc.sync.dma_start(
                        wkr_p[:], d_wkr.rearrange("(k p) n -> p k n", p=128)[:, :, p * 128:(p + 1) * 128]
                    )
                    nc.sync.dma_start(
                        wv_p[:], d_wv.rearrange("(k p) n -> p k n", p=128)[:, :, p * 384:(p + 1) * 384]
                    )
                    if p % 2 == 0:
                        wo_p = wp.tile([128, 6, C], BF, tag="wo_p", bufs=1)
                        nc.sync.dma_start(
                            wo_p[:], d_wo.rearrange("(k p) n -> p k n", p=128)[:, 6 * (p // 2):6 * (p // 2) + 6, :]
                        )
                        attn2 = hw.tile([128, 6, TQ], BF, tag="attn")
                    attn = attn2[:, 3 * (p % 2):3 * (p % 2) + 3, :]

                    # --- up-projections: all q-side first (q_lat only), so
                    # pair 0's PE work covers the kv AllGather tail ---
                    qc = []
                    kc = []
                    for w_ in range(2):
                        psq2 = psmm.tile([128, 512], F32, tag="mm", bufs=3)
                        for kt in range(12):
                            nc.tensor.matmul(
                                psq2[:],
                                lhsT=wqn_p[:, kt, w_ * 128:(w_ + 1) * 128],
                                rhs=q_lat[:, kt, :],
                                start=(kt == 0),
                                stop=(kt == 11),
                            )
                        qc_w = hw.tile([128, TQ], BF, tag=f"qc{w_}")
                        nc.vector.tensor_copy(qc_w[:], psq2[:])
                        qc.append(qc_w)
                    for w_ in range(2):
                        kc_w = hw.tile([128, T], BF, tag=f"kc{w_}")
                        for tch in range(2):
                            psk2 = psmm.tile([128, 512], F32, tag="mm", bufs=3)
                            for kt in range(4):
                                nc.tensor.matmul(
                                    psk2[:],
                                    lhsT=wkn_p[:, kt, w_ * 128:(w_ + 1) * 128],
                                    rhs=kv_lat[:, kt, tch * 512:(tch + 1) * 512],
                                    start=(kt == 0),
                                    stop=(kt == 3),
                                )
                            nc.vector.tensor_copy(kc_w[:, tch * 512:(tch + 1) * 512], psk2[:])
                        kc.append(kc_w)

                    # --- rope: q (both heads of pair share the [128, TQ] tile) ---
                    psr = psmm.tile([128, 512], F32, tag="mm", bufs=3)
                    for kt in range(12):
                        nc.tensor.matmul(
                            psr[:],
                            lhsT=wqr_p[:, kt, :],
                            rhs=q_lat[:, kt, :],
                            start=(kt == 0),
                            stop=(kt == 11),
                        )
                    qshf = hw.tile([128, TQ], F32, tag="qshf", bufs=1)
                    nc.vector.stream_shuffle(qshf[:], psr[:], SHUF)
                    qro = hw.tile([128, TQ], BF, tag="qro")
                    qtmp = hw.tile([128, TQ], BF, tag="qtmp")
                    nc.vector.tensor_tensor(qro[:], psr[:], cq[:], MULT)
                    nc.vector.tensor_tensor(qtmp[:], qshf[:], sq[:], MULT)
                    nc.vector.tensor_add(qro[:], qro[:], qtmp[:])

                    # --- rope: k ---
                    kro = hw.tile([128, T], BF, tag="kro")
                    kshf = hw.tile([128, T], F32, tag="kshf", bufs=1)
                    ktmp = hw.tile([128, T], BF, tag="ktmp")
                    for tch in range(2):
                        sl = slice(tch * 512, (tch + 1) * 512)
                        psr2 = psmm.tile([128, 512], F32, tag="mm", bufs=3)
                        for kt in range(4):
                            nc.tensor.matmul(
                                psr2[:],
                                lhsT=wkr_p[:, kt, :],
                                rhs=kv_lat[:, kt, tch * 512:(tch + 1) * 512],
                                start=(kt == 0),
                                stop=(kt == 3),
                            )
                        nc.vector.stream_shuffle(kshf[:, sl], psr2[:], SHUF)
                        nc.vector.tensor_tensor(kro[:, sl], psr2[:], ck[:, sl], MULT)
                    nc.vector.tensor_tensor(ktmp[:], kshf[:], sk[:], MULT)
                    nc.vector.tensor_add(kro[:], kro[:], ktmp[:])

                    # --- v: [he d0:192 | ones_e@192 | ones_o@193 | ho d0:192 @194:386] ---
                    v_pr = hw.tile([128, 8, 386], BF, tag="v_pr", bufs=3)
                    for st in range(NST):
                        psv = psmm.tile([128, 384], F32, tag="mm", bufs=3)
                        for kt in range(4):
                            nc.tensor.matmul(
                                psv[:],
                                lhsT=kv_lat[:, kt, st * 128:(st + 1) * 128],
                                rhs=wv_p[:, kt, :],
                                start=(kt == 0),
                                stop=(kt == 3),
                            )
                        nc.vector.tensor_copy(v_pr[:, st, 0:192], psv[:, 0:192])
                        nc.vector.tensor_copy(v_pr[:, st, 194:386], psv[:, 192:384])
                    nc.vector.memset(v_pr[:, :, 192:194], 1.0)

                    # --- attention for both heads of the pair ---
                    for w_ in range(2):
                        psA = pspv.tile([128, 512], F32, tag="psA")
                        psB = pspv.tile([128, 512], F32, tag="psB")
                        for st in range(NST):
                            wide = st in WIDE_STS
                            N = 512 if wide else 256
                            c0 = 0 if wide else 256
                            csl = slice(c0, 512)
                            pss = pssp.tile([128, 512], F32, tag="pss")
                            nc.tensor.matmul(
                                pss[:, 0:N],
                                lhsT=kc[w_][:, st * 128:(st + 1) * 128],
                                rhs=qc[w_][:, csl],
                                start=True,
                                stop=False,
                            )
                            nc.tensor.matmul(
                                pss[:, 0:N],
                                lhsT=kro[w_ * 64:(w_ + 1) * 64, st * 128:(st + 1) * 128],
                                rhs=qro[w_ * 64:(w_ + 1) * 64, csl],
                                start=False,
                                stop=True,
                            )
                            pr = prp.tile([128, 512], BF, tag="pr")
                            nc.scalar.activation(pr[:, 0:N], pss[:, 0:N], EXP, scale=SCALE)
                            if wide:
                                nc.vector.tensor_tensor(
                                    pr[:, 0:N], pr[:, 0:N],
                                    mAB[:, WI[st] * 512:(WI[st] + 1) * 512], MULT,
                                )
                            else:
                                nc.vector.tensor_tensor(
                                    pr[:, 0:N], pr[:, 0:N],
                                    mAB[:, 8 * CH + NI[st] * CH:8 * CH + (NI[st] + 1) * CH], MULT,
                                )
                            # PV accumulate; q-block A [0:256] gets s-tiles
                            # {0,1,4,5} (stop at 5), block B [256:512] gets all
                            # 8 (stop at 7)
                            first = st == 0
                            if st == 5:
                                pv_parts = [(slice(0, 256), slice(0, 256), True),
                                            (slice(256, 512), slice(256, 512), False)]
                            elif st == 7:
                                pv_parts = [(slice(256, 512), slice(0, 256), True)]
                            else:
                                pv_parts = [(slice(c0, 512), slice(0, N), False)]
                            for dcols, prcols, stop_f in pv_parts:
                                if w_ == 0:
                                    nc.tensor.matmul(
                                        psA[0:128, dcols], lhsT=v_pr[:, st, 0:128],
                                        rhs=pr[:, prcols], start=first, stop=stop_f, skip_group_check=True,
                                    )
                                    nc.tensor.matmul(
                                        psB[0:65, dcols], lhsT=v_pr[:, st, 128:193],
                                        rhs=pr[:, prcols], start=first, stop=stop_f, skip_group_check=True,
                                    )
                                else:
                                    nc.tensor.matmul(
                                        psA[32:33, dcols], lhsT=v_pr[:, st, 193:194],
                                        rhs=pr[:, prcols], start=first, stop=stop_f, skip_group_check=True,
                                    )
                                    nc.tensor.matmul(
                                        psA[64:128, dcols], lhsT=v_pr[:, st, 194:258],
                                        rhs=pr[:, prcols], start=first, stop=stop_f, skip_group_check=True,
                                    )
                                    nc.tensor.matmul(
                                        psB[0:128, dcols], lhsT=v_pr[:, st, 258:386],
                                        rhs=pr[:, prcols], start=first, stop=stop_f, skip_group_check=True,
                                    )
                        # normalize + evict into attn ([f%128, ft, tq])
                        k0 = w_  # pair-local f-blocks: even head (0,1), odd head (1,2)
                        r_sb = smp.tile([1, 512], F32, tag="r_sb")
                        denom = psB[64:65, :] if w_ == 0 else psA[32:33, :]
                        nc.vector.reciprocal(r_sb[:], denom)
                        Rb = smp.tile([128, 512], F32, tag="Rb")
                        nc.gpsimd.partition_broadcast(Rb[:], r_sb[:])
                        if w_ == 0:
                            nc.vector.tensor_tensor(
                                attn[0:128, k0, :], psA[0:128, :], Rb[0:128, :], MULT
                            )
                            nc.vector.tensor_tensor(
                                attn[0:64, k0 + 1, :], psB[0:64, :], Rb[0:64, :], MULT
                            )
                        else:
                            nc.vector.tensor_tensor(
                                attn[64:128, k0, :], psA[64:128, :], Rb[64:128, :], MULT
                            )
                            nc.vector.tensor_tensor(
                                attn[0:128, k0 + 1, :], psB[0:128, :], Rb[0:128, :], MULT
                            )

                    if p % 2 == 1:
                        emit_wo(p // 2, attn2, wo_p)

    nc.compile()
    return nc


# ---------------- host-side preparation ----------------

def _tq_cols(j):
    if j == 0:
        return np.concatenate([np.arange(0, 256), np.arange(768, 1024)])
    return np.arange(256, 768)


# kv/s global token order after the pair AllGather: even core's chunks (0,3)
# then odd core's chunks (1,2)
S_PERM = np.concatenate([_tq_cols(0), _tq_cols(1)])


def _rope_tables():
    inv = ROPE_BASE ** (-np.arange(0, DR, 2, dtype=np.float64) / DR)  # [32]
    t = np.arange(T, dtype=np.float64)
    ang = np.outer(t, inv)  # [T, 32]
    cosT = np.cos(ang).T.astype(np.float32)  # [32, T]
    sinT = np.sin(ang).T.astype(np.float32)
    # row r (mod 64): b2 = (r%64)//32, pos = r%32
    # pos<16 -> x1 of freq b2*16+pos (sign -), else x2 of freq b2*16+pos-16 (sign +)
    Ck = np.empty((128, T), np.float32)
    Sk = np.empty((128, T), np.float32)
    for r in range(128):
        rr = r % 64
        b2, pos = rr // 32, rr % 32
        if pos < 16:
            f = b2 * 16 + pos
            Ck[r], Sk[r] = cosT[f], -sinT[f]
        else:
            f = b2 * 16 + pos - 16
            Ck[r], Sk[r] = cosT[f], sinT[f]
    return Ck, Sk


_ROPE_PERM = []
for _b2 in range(2):
    _ROPE_PERM += [2 * (16 * _b2 + i) for i in range(16)]       # x1 rows
    _ROPE_PERM += [2 * (16 * _b2 + i) + 1 for i in range(16)]   # x2 rows


def _deinterleave_cols(w):
    # per head: rows [x1 f0..15 | x2 f0..15 | x1 f16..31 | x2 f16..31]
    r = w.shape[0]
    wh = w.reshape(r, H, DR)
    return wh[:, :, _ROPE_PERM].reshape(r, H * DR)


def _mask_deltas(j):
    """Per-block thresholds: mask_block[r, c] = (r - c <= delta).

    Block (st, q-chunk) is valid where S_PERM[st*128] + r <= t_base + c,
    i.e. r - c <= t_base - s_base. Clamped to +-256 (exact in bf16) since
    rc spans [-255, 127].
    """
    own = _tq_cols(j)
    t_base = {"A": own[0], "B": own[256]}
    deltas = []
    for st in WIDE_STS:
        s_base = S_PERM[st * 128]
        deltas += [t_base["A"] - s_base, t_base["B"] - s_base]
    for st in NARROW_STS:
        s_base = S_PERM[st * 128]
        deltas += [t_base["B"] - s_base]
    d = np.clip(np.array(deltas, np.float32), -256, 256)
    d = np.concatenate([d, [1.0 - j, float(j)]])  # q-table half-select
    return np.ascontiguousarray(d[None, :])  # [1, 14]


def _static_tables():
    return [{"dl": _mask_deltas(c % 2)} for c in range(NCORES)]


_STATIC = _static_tables()


def prepare_weights(inputs):
    return {
        "wqd": np.asarray(inputs["wq_down"], np.float32).astype(NPBF),
        "wkd": np.asarray(inputs["wkv_down"], np.float32).astype(NPBF),
        "wqn": np.asarray(inputs["wq_nope"], np.float32).astype(NPBF),
        "wqr": _deinterleave_cols(np.asarray(inputs["wq_rope"], np.float32)).astype(NPBF),
        "wkn": np.asarray(inputs["wk_nope"], np.float32).astype(NPBF),
        "wkr": _deinterleave_cols(np.asarray(inputs["wk_rope"], np.float32)).astype(NPBF),
        "wv": np.asarray(inputs["wv_up"], np.float32).astype(NPBF),
        "wo": np.asarray(inputs["wo"], np.float32).astype(NPBF),
    }


def make_in_maps(inputs):
    x = np.asarray(inputs["x"], np.float32)
    in_maps = []
    for c in range(NCORES):
        b, j = c // 2, c % 2
        cols = _tq_cols(j)
        xT = np.ascontiguousarray(x[b].T[:, cols].astype(NPBF))  # [C, TQ]
        in_maps.append({"xq": xT, **_STATIC[c]})
    return in_maps


def assemble_output(results):
    out = np.empty((B, T, C), np.float32)
    for c in range(NCORES):
        b, j = c // 2, c % 2
        out[b, _tq_cols(j), :] = np.asarray(results[c]["out"]).astype(np.float32)
    return out


def _weights_key(inputs):
    h = hashlib.blake2b(digest_size=16)
    for k in W_NAMES:
        a = np.ascontiguousarray(np.asarray(inputs[k]))
        h.update(a.tobytes())
    return h.digest()


def kernel(**inputs):
    global _CACHED_NC, _CACHED_KEY
    key = _weights_key(inputs)
    if _CACHED_NC is None or key != _CACHED_KEY:
        _CACHED_NC = build_nc(prepare_weights(inputs))
        _CACHED_KEY = key
    in_maps = make_in_maps(inputs)
    trace = bool(int(os.environ.get("MLA_TRACE", "0")))
    try:
        res = run_bass_kernel_spmd(
            _CACHED_NC, in_maps, core_ids=list(range(NCORES)), trace=trace
        )
    except ModuleNotFoundError:
        # no NTFF profiling hook in this environment -> run untraced
        res = run_bass_kernel_spmd(
            _CACHED_NC, in_maps, core_ids=list(range(NCORES)), trace=False
        )
    out = assemble_output(res.results)
    if trace:
        kernel.last_exec_time_ns = res.exec_time_ns
        kernel.last_results = res
    return out
